# revision 26
# baseline (speedup 1.0000x reference)
"""Trainium2 Bass kernel for the 3-omics GNN encoder (gnn_message_passing).

Math (per reference):
    adj_i   = w0_i*A_sp_i + w1_i*A_ft_i + b_i            (dense fused adjacency)
    lat_i   = adj_i @ (feat_i @ W_enc_i)                 [N, 64]
    combined= (cat(lat) @ mlp_w1 + b1) @ mlp_w2 + b2     [N, 64]
    rec_i   = A_sp_i @ combined @ W_dec_i                [N, D]

Key algebraic optimizations (exact in exact arithmetic):
  * rec_i is computed as (A_sp_i @ combined) @ W_dec_i  -- drops ~216 GFLOP
    per decoder to ~7 GFLOP and makes the problem HBM-bound.
  * the fused adjacency is never materialized:
      lat_i = A_sp_i @ (feat_i @ (w0_i*W_enc_i))
            + A_ft_i @ (feat_i @ (w1_i*W_enc_i))
            + colsum(feat_i @ (b_i*W_enc_i))   (broadcast over rows)
    The three scaled weight copies are concatenated host-side into one
    [D, 192] "wcat" so one matmul pass produces all three Y blocks.

Sharding: rows (nodes) are split across the 8 cores (750 rows each).  Each
core receives its row-slice of features / adjacencies pre-transposed by the
host so every device matmul uses natural (un-transposed) SBUF layouts.  The
small [N, 192] Y and [N, 64] combined activations are all-gathered on-device.
"""

import numpy as np

import concourse.bass as bass
from concourse import bacc
import concourse.mybir as mybir
import concourse.tile as tile
from concourse.bass_utils import run_bass_kernel_spmd

F32 = mybir.dt.float32
F32R = mybir.dt.float32r
F16 = mybir.dt.float16

N_FULL = 6000
D_FULL = 3000
DO = 64
N_CORES = 8


def _chunks(total, step):
    """[(offset, size), ...] covering `total` in steps of `step`."""
    return [(o, min(step, total - o)) for o in range(0, total, step)]


def _pair_groups(chunks):
    """Group consecutive full-128 tiles in pairs (for 2x-sized DMAs)."""
    groups, ii = [], 0
    while ii < len(chunks):
        if (chunks[ii][1] == 128 and ii + 1 < len(chunks)
                and chunks[ii + 1][1] == 128):
            groups.append([(ii, *chunks[ii]), (ii + 1, *chunks[ii + 1])])
            ii += 2
        else:
            groups.append([(ii, *chunks[ii])])
            ii += 1
    return groups


def build_nc(n=N_FULL, d=D_FULL, n_cores=N_CORES, rc_step=384, nd_step=512):
    """Build the SPMD Bass program (identical on every core)."""
    r = n // n_cores  # rows per core
    nc = bacc.Bacc(num_devices=n_cores, num_swdge_queues=4)

    kN = _chunks(n, 128)      # contraction tiles over the node dim
    kD = _chunks(d, 128)      # contraction tiles over the feature dim
    mR = _chunks(r, 128)      # output row tiles (partition dim)
    rcR = _chunks(r, rc_step)  # row chunks (moving free dim)
    nD = _chunks(d, nd_step)   # rec output column chunks

    # ---- I/O ----------------------------------------------------------
    ft = [nc.dram_tensor(f"ft{i}", [d, r], F16, kind="ExternalInput")
          for i in range(3)]
    aspT = [nc.dram_tensor(f"aspT{i}", [n, r], F16, kind="ExternalInput")
            for i in range(3)]
    aftT = [nc.dram_tensor(f"aftT{i}", [n, r], F16, kind="ExternalInput")
            for i in range(3)]
    WC = 3 * DO  # fp16 matmuls run 1 cyc/row at any width: no padding needed
    wcat = [nc.dram_tensor(f"wcat{i}", [d, WC], F16, kind="ExternalInput")
            for i in range(3)]
    wdec = [nc.dram_tensor(f"wdec{i}", [DO, d], F32R, kind="ExternalInput")
            for i in range(3)]
    mw1 = nc.dram_tensor("mw1", [3 * DO, DO], F32R, kind="ExternalInput")
    mb1 = nc.dram_tensor("mb1", [DO, 1], F32, kind="ExternalInput")
    mw2 = nc.dram_tensor("mw2", [DO, DO], F32R, kind="ExternalInput")
    mb2b = nc.dram_tensor("mb2b", [128, DO], F32, kind="ExternalInput")

    latT_out = [nc.dram_tensor(f"latT{i}", [DO, r], F32R, kind="ExternalOutput")
                for i in range(3)]
    comb_out = nc.dram_tensor("comb", [r, DO], F32R, kind="ExternalOutput")
    rec_out = [nc.dram_tensor(f"rec{i}", [r, d], F32R, kind="ExternalOutput")
               for i in range(3)]

    rg = [list(range(n_cores))]

    with tile.TileContext(nc) as tc:
        with (
            tc.tile_pool(name="dram", bufs=1, space="DRAM") as dram,
            tc.tile_pool(name="consts", bufs=1) as consts,
            tc.tile_pool(name="wcat_p", bufs=24) as wcat_p,
            tc.tile_pool(name="fstrip_p", bufs=6) as fstrip_p,
            tc.tile_pool(name="ysb_p", bufs=3) as ysb_p,
            tc.tile_pool(name="ycat_p", bufs=1) as ycat_p,
            tc.tile_pool(name="astrip_p", bufs=8) as astrip_p,
            tc.tile_pool(name="lat_p", bufs=3) as lat_p,
            tc.tile_pool(name="mlp_p", bufs=1) as mlp_p,
            tc.tile_pool(name="csb_p", bufs=4) as csb_p,
            tc.tile_pool(name="cf_p", bufs=1) as cf_p,
            tc.tile_pool(name="wdec_p", bufs=2) as wdec_p,
            tc.tile_pool(name="tsb_p", bufs=2) as tsb_p,
            tc.tile_pool(name="rsb_p", bufs=6) as rsb_p,
            tc.tile_pool(name="psum", bufs=6, space="PSUM") as psum,
            tc.tile_pool(name="psum_s", bufs=2, space="PSUM") as psum_s_p,
        ):
            # internal DRAM for the collectives
            yloc = [dram.tile([r, 3 * DO], F16, name=f"yloc{i}", tag=f"yloc{i}")
                    for i in range(3)]
            yfull = [dram.tile([n, 3 * DO], F16, name=f"yfull{i}",
                               tag=f"yfull{i}", addr_space="Shared")
                     for i in range(3)]
            cloc = dram.tile([r, DO], F16, name="cloc", tag="cloc")
            cfull = dram.tile([n, DO], F16, name="cfull", tag="cfull",
                              addr_space="Shared")

            ones_col = consts.tile([128, 1], F32, name="ones_col")
            nc.vector.memset(ones_col[:], 1.0)

            # ============ Phase A: Y_i = feat_i @ wcat_i, all-gather ======
            for i in range(3):
                wcat_t = []
                for kt, (ko, ksz) in enumerate(kD):
                    wt = wcat_p.tile([128, WC], F16, name="wt",
                                     tag="wcat" if ksz == 128 else "wcat_t",
                                     bufs=24 if ksz == 128 else 2)
                    nc.scalar.dma_start(wt[:ksz, :], wcat[i][ko:ko + ksz, :])
                    wcat_t.append(wt)
                psum_y = [psum.tile([128, WC], F32, name=f"psum_y{m}",
                                    tag="big")
                          for m in range(len(mR))]
                for grp in _pair_groups(kD):
                    if len(grp) == 2:
                        fstrip = fstrip_p.tile([128, 2, r], F16,
                                               name="fstrip", tag="fs2",
                                               bufs=4)
                        nc.gpsimd.dma_start(
                            fstrip[:, :, :],
                            ft[i][grp[0][1]:grp[0][1] + 256, :].rearrange(
                                "(two p) q -> p two q", p=128))
                    else:
                        ksz0 = grp[0][2]
                        fstrip = fstrip_p.tile(
                            [128, 1, r], F16, name="fstrip",
                            tag="fs1" if ksz0 == 128 else "fs_t", bufs=2)
                        nc.gpsimd.dma_start(
                            fstrip[:ksz0, 0, :],
                            ft[i][grp[0][1]:grp[0][1] + ksz0, :])
                    for t, (kt, ko, ksz) in enumerate(grp):
                        for m, (mo, msz) in enumerate(mR):
                            nc.tensor.matmul(
                                psum_y[m][:msz, :],
                                fstrip[:ksz, t, mo:mo + msz],
                                wcat_t[kt][:ksz, :],
                                start=(kt == 0),
                                stop=(kt == len(kD) - 1),
                            )  # psum cols 192:256 are zero padding
                for m, (mo, msz) in enumerate(mR):
                    y_sb = ysb_p.tile([128, 3 * DO], F16, name="y_sb",
                                      tag="y_sb")
                    nc.vector.tensor_copy(out=y_sb[:msz, :],
                                          in_=psum_y[m][:msz, :])
                    nc.scalar.dma_start(yloc[i][mo:mo + msz, :], y_sb[:msz, :])
                nc.gpsimd.collective_compute(
                    "AllGather", mybir.AluOpType.bypass, replica_groups=rg,
                    ins=[yloc[i][:].opt()], outs=[yfull[i][:].opt()],
                )

            # ============ Phase B: latT_i, then MLP -> combined ==========
            lat_sb = []
            for i in range(3):
                ycat_sb = ycat_p.tile([128, len(kN), 3 * DO], F16,
                                      name="ycat_sb", tag="ycat_sb")
                nfull, ntail = n // 128, n % 128
                nc.gpsimd.dma_start(
                    ycat_sb[:, :nfull, :],
                    yfull[i][:nfull * 128, :].rearrange(
                        "(kt p) c -> p kt c", p=128))
                if ntail:
                    zbase = (ntail // 32) * 32
                    nc.vector.memset(ycat_sb[zbase:, nfull, :], 0.0)
                    nc.gpsimd.dma_start(ycat_sb[:ntail, nfull, :],
                                        yfull[i][nfull * 128:, :])
                # column-sum of the b-scaled Y block: DVE-reduce over the
                # k-tile axis, then one ones-matmul to reduce partitions.
                ysum = consts.tile([128, DO], F32, name=f"ysum{i}",
                                   tag=f"ysum{i}")
                nc.vector.reduce_sum(
                    ysum[:, :],
                    ycat_sb[:, :, 2 * DO:3 * DO].rearrange("p k c -> p c k"),
                    axis=mybir.AxisListType.X)
                ps_s = psum_s_p.tile([DO, 1], F32, name="ps_s", tag="s")
                nc.tensor.matmul(ps_s[:, :], ysum[:, :], ones_col[:, :],
                                 start=True, stop=True)
                ps_lat = [psum.tile([DO, 512], F32, name=f"ps_lat{rc}",
                                    tag="big")[:, :rcsz]
                          for rc, (rco, rcsz) in enumerate(rcR)]
                for grp in _pair_groups(kN):
                    if len(grp) == 2:
                        sa = astrip_p.tile([128, 2, r], F16, name="sa",
                                           tag="as2", bufs=6)
                        nc.sync.dma_start(
                            sa[:, :, :],
                            aspT[i][grp[0][1]:grp[0][1] + 256, :].rearrange(
                                "(two p) q -> p two q", p=128))
                        sf = astrip_p.tile([128, 2, r], F16, name="sf",
                                           tag="af2", bufs=6)
                        nc.sync.dma_start(
                            sf[:, :, :],
                            aftT[i][grp[0][1]:grp[0][1] + 256, :].rearrange(
                                "(two p) q -> p two q", p=128))
                    else:
                        ksz0 = grp[0][2]
                        stag = "1" if ksz0 == 128 else "_t"
                        sa = astrip_p.tile([128, 1, r], F16, name="sa",
                                           tag="as" + stag, bufs=2)
                        nc.sync.dma_start(
                            sa[:ksz0, 0, :],
                            aspT[i][grp[0][1]:grp[0][1] + ksz0, :])
                        sf = astrip_p.tile([128, 1, r], F16, name="sf",
                                           tag="af" + stag, bufs=2)
                        nc.sync.dma_start(
                            sf[:ksz0, 0, :],
                            aftT[i][grp[0][1]:grp[0][1] + ksz0, :])
                    for t, (kt, ko, ksz) in enumerate(grp):
                        y0 = ycat_sb[:ksz, kt, 0:DO]
                        y1 = ycat_sb[:ksz, kt, DO:2 * DO]
                        for rc, (rco, rcsz) in enumerate(rcR):
                            nc.tensor.matmul(ps_lat[rc], y0,
                                             sa[:ksz, t, rco:rco + rcsz],
                                             start=(kt == 0), stop=False)
                        for rc, (rco, rcsz) in enumerate(rcR):
                            nc.tensor.matmul(ps_lat[rc], y1,
                                             sf[:ksz, t, rco:rco + rcsz],
                                             start=False,
                                             stop=(kt == len(kN) - 1))
                s_col = consts.tile([DO, 1], F32, name=f"s_col{i}",
                                    tag=f"s_col{i}")
                nc.vector.tensor_copy(out=s_col[:], in_=ps_s[:, :])
                lsb = lat_p.tile([DO, r], F32R, name=f"lat_sb{i}", tag=f"lat{i}")
                for rc, (rco, rcsz) in enumerate(rcR):
                    nc.vector.tensor_tensor(
                        lsb[:, rco:rco + rcsz], ps_lat[rc],
                        s_col[:].to_broadcast([DO, rcsz]),
                        mybir.AluOpType.add,
                    )
                nc.scalar.dma_start(latT_out[i][:, :], lsb[:, :])
                lat_sb.append(lsb)

            # ---- MLP ----
            mw1_sb = mlp_p.tile([DO, 3, DO], F32R, name="mw1_sb", tag="mw1")
            for i in range(3):
                nc.sync.dma_start(mw1_sb[:, i, :], mw1[i * DO:(i + 1) * DO, :])
            mb1_sb = mlp_p.tile([DO, 1], F32, name="mb1_sb", tag="mb1")
            nc.sync.dma_start(mb1_sb[:, :], mb1[:, :])
            mw2_sb = mlp_p.tile([DO, DO], F32R, name="mw2_sb", tag="mw2")
            nc.sync.dma_start(mw2_sb[:, :], mw2[:, :])
            mb2_sb = mlp_p.tile([128, DO], F32, name="mb2_sb", tag="mb2")
            nc.sync.dma_start(mb2_sb[:, :], mb2b[:, :])

            hT_sb = mlp_p.tile([DO, r], F32R, name="hT_sb", tag="hT")
            for rc, (rco, rcsz) in enumerate(rcR):
                ps_h = psum.tile([DO, 512], F32, name="ps_h", tag="big")
                for i in range(3):
                    nc.tensor.matmul(ps_h[:, :rcsz], mw1_sb[:, i, :],
                                     lat_sb[i][:, rco:rco + rcsz],
                                     start=(i == 0), stop=(i == 2))
                nc.vector.tensor_tensor(
                    hT_sb[:, rco:rco + rcsz], ps_h[:, :rcsz],
                    mb1_sb[:].to_broadcast([DO, rcsz]),
                    mybir.AluOpType.add,
                )
            for m, (mo, msz) in enumerate(mR):
                ps_c = psum.tile([128, DO], F32, name="ps_c", tag="big")
                nc.tensor.matmul(ps_c[:msz, :], hT_sb[:, mo:mo + msz],
                                 mw2_sb[:, :], start=True, stop=True)
                c_sb = csb_p.tile([128, DO], F32R, name="c_sb", tag="c_sb")
                nc.vector.tensor_tensor(c_sb[:msz, :], ps_c[:msz, :],
                                        mb2_sb[:msz, :], mybir.AluOpType.add)
                c16 = csb_p.tile([128, DO], F16, name="c16", tag="c16")
                nc.vector.tensor_copy(out=c16[:msz, :], in_=c_sb[:msz, :])
                nc.scalar.dma_start(comb_out[mo:mo + msz, :], c_sb[:msz, :])
                nc.scalar.dma_start(cloc[mo:mo + msz, :], c16[:msz, :])
            nc.gpsimd.collective_compute(
                "AllGather", mybir.AluOpType.bypass, replica_groups=rg,
                ins=[cloc[:].opt()], outs=[cfull[:].opt()],
            )

            # ============ Phase C: rec_i = (A_sp_i @ combined) @ W_dec_i ==
            cf_sb = cf_p.tile([128, len(kN), DO], F16, name="cf_sb", tag="cf")
            nfull, ntail = n // 128, n % 128
            nc.scalar.dma_start(
                cf_sb[:, :nfull, :],
                cfull[:nfull * 128, :].rearrange("(kt p) c -> p kt c", p=128))
            if ntail:
                nc.scalar.dma_start(cf_sb[:ntail, nfull, :],
                                    cfull[nfull * 128:, :])
            for i in range(3):
                wdec_sb = wdec_p.tile([DO, d], F32R, name="wdec_sb", tag="wdec")
                nc.scalar.dma_start(wdec_sb[:, :], wdec[i][:, :])
                ps_t = [psum.tile([DO, 512], F32, name=f"ps_t{rc}",
                                  tag="big")[:, :rcsz]
                        for rc, (rco, rcsz) in enumerate(rcR)]
                for gi, grp in enumerate(_pair_groups(kN)):
                    eng = nc.sync if gi % 2 == 0 else nc.scalar
                    if len(grp) == 2:
                        st = astrip_p.tile([128, 2, r], F16, name="st",
                                           tag="as2", bufs=6)
                        eng.dma_start(
                            st[:, :, :],
                            aspT[i][grp[0][1]:grp[0][1] + 256, :].rearrange(
                                "(two p) q -> p two q", p=128))
                    else:
                        ksz0 = grp[0][2]
                        st = astrip_p.tile(
                            [128, 1, r], F16, name="st",
                            tag="as1" if ksz0 == 128 else "as_t", bufs=2)
                        eng.dma_start(st[:ksz0, 0, :],
                                      aspT[i][grp[0][1]:grp[0][1] + ksz0, :])
                    for t, (kt, ko, ksz) in enumerate(grp):
                        for rc, (rco, rcsz) in enumerate(rcR):
                            nc.tensor.matmul(ps_t[rc], cf_sb[:ksz, kt, :],
                                             st[:ksz, t, rco:rco + rcsz],
                                             start=(kt == 0),
                                             stop=(kt == len(kN) - 1))
                t_sb = tsb_p.tile([DO, r], F32R, name="t_sb", tag="t_sb")
                for rc, (rco, rcsz) in enumerate(rcR):
                    nc.vector.tensor_copy(out=t_sb[:, rco:rco + rcsz],
                                          in_=ps_t[rc])
                for m, (mo, msz) in enumerate(mR):
                    for nck, (no, nsz) in enumerate(nD):
                        ps_r = psum.tile([128, 512], F32, name="ps_r",
                                         tag="big")
                        nc.tensor.matmul(ps_r[:msz, :nsz],
                                         t_sb[:, mo:mo + msz],
                                         wdec_sb[:, no:no + nsz],
                                         start=True, stop=True)
                        r_sb = rsb_p.tile([128, 512], F32R, name="r_sb",
                                          tag="r_sb")
                        nc.vector.tensor_copy(out=r_sb[:msz, :nsz],
                                              in_=ps_r[:msz, :nsz])
                        nc.gpsimd.dma_start(rec_out[i][mo:mo + msz, no:no + nsz],
                                          r_sb[:msz, :nsz])
    if not nc.is_finalized():
        nc.finalize()
    return nc


def _prep_in_maps(inputs, n, d, n_cores):
    """Host-side sharding: row-slice + transpose the big operands."""
    r = n // n_cores
    feats = [inputs["features_omics1"], inputs["features_omics2"],
             inputs["features_omics3"]]
    asp = [inputs["adj_spatial_omics1"], inputs["adj_spatial_omics2"],
           inputs["adj_spatial_omics3"]]
    aft = [inputs["adj_feature_omics1"], inputs["adj_feature_omics2"],
           inputs["adj_feature_omics3"]]
    convw = [inputs["conv_w1"], inputs["conv_w2"], inputs["conv_w3"]]
    convb = [inputs["conv_b1"], inputs["conv_b2"], inputs["conv_b3"]]
    wenc = [inputs["W_enc1"], inputs["W_enc2"], inputs["W_enc3"]]
    wdec = [inputs["W_dec1"], inputs["W_dec2"], inputs["W_dec3"]]

    f32 = np.float32
    shared = {}
    for i in range(3):
        w0 = f32(np.asarray(convw[i])[0])
        w1 = f32(np.asarray(convw[i])[1])
        b = f32(np.asarray(convb[i]))
        W = np.asarray(wenc[i], f32)
        shared[f"wcat{i}"] = np.ascontiguousarray(
            np.concatenate([w0 * W, w1 * W, b * W], axis=1)).astype(np.float16)
        shared[f"wdec{i}"] = np.ascontiguousarray(np.asarray(wdec[i], f32))
    shared["mw1"] = np.ascontiguousarray(np.asarray(inputs["mlp_w1"], f32))
    shared["mb1"] = np.ascontiguousarray(
        np.asarray(inputs["mlp_b1"], f32).reshape(DO, 1))
    shared["mw2"] = np.ascontiguousarray(np.asarray(inputs["mlp_w2"], f32))
    shared["mb2b"] = np.ascontiguousarray(
        np.tile(np.asarray(inputs["mlp_b2"], f32).reshape(1, DO), (128, 1)))

    in_maps = []
    for c in range(n_cores):
        sl = slice(c * r, (c + 1) * r)
        m = dict(shared)
        for i in range(3):
            m[f"ft{i}"] = np.ascontiguousarray(np.asarray(feats[i][sl], f32).T).astype(np.float16)
            m[f"aspT{i}"] = np.ascontiguousarray(
                np.asarray(asp[i][sl], f32).T).astype(np.float16)
            m[f"aftT{i}"] = np.ascontiguousarray(
                np.asarray(aft[i][sl], f32).T).astype(np.float16)
        in_maps.append(m)
    return in_maps


_NC_CACHE = {}


def kernel(_trace=False, **inputs):
    n, d, n_cores = N_FULL, D_FULL, N_CORES
    in_maps = _prep_in_maps(inputs, n, d, n_cores)
    key = (n, d, n_cores)
    if key not in _NC_CACHE:
        _NC_CACHE[key] = build_nc(n, d, n_cores)
    nc = _NC_CACHE[key]
    res = run_bass_kernel_spmd(nc, in_maps, core_ids=list(range(n_cores)),
                               trace=_trace)
    if _trace:
        kernel._last_results = res
    outs = res.results
    lat = [np.concatenate([outs[c][f"latT{i}"].T for c in range(n_cores)],
                          axis=0)
           for i in range(3)]
    comb = np.concatenate([outs[c]["comb"] for c in range(n_cores)], axis=0)
    rec = [np.concatenate([outs[c][f"rec{i}"] for c in range(n_cores)], axis=0)
           for i in range(3)]
    return (np.ascontiguousarray(lat[0], np.float32),
            np.ascontiguousarray(lat[1], np.float32),
            np.ascontiguousarray(lat[2], np.float32),
            np.ascontiguousarray(comb, np.float32),
            np.ascontiguousarray(rec[0], np.float32),
            np.ascontiguousarray(rec[1], np.float32),
            np.ascontiguousarray(rec[2], np.float32))


# revision 27
# speedup vs baseline: 1.0171x; 1.0171x over previous
"""Trainium2 Bass kernel for the 3-omics GNN encoder (gnn_message_passing).

Math (per reference):
    adj_i   = w0_i*A_sp_i + w1_i*A_ft_i + b_i            (dense fused adjacency)
    lat_i   = adj_i @ (feat_i @ W_enc_i)                 [N, 64]
    combined= (cat(lat) @ mlp_w1 + b1) @ mlp_w2 + b2     [N, 64]
    rec_i   = A_sp_i @ combined @ W_dec_i                [N, D]

Key algebraic optimizations (exact in exact arithmetic):
  * rec_i is computed as (A_sp_i @ combined) @ W_dec_i  -- drops ~216 GFLOP
    per decoder to ~7 GFLOP and makes the problem HBM-bound.
  * the fused adjacency is never materialized:
      lat_i = A_sp_i @ (feat_i @ (w0_i*W_enc_i))
            + A_ft_i @ (feat_i @ (w1_i*W_enc_i))
            + colsum(feat_i @ (b_i*W_enc_i))   (broadcast over rows)
    The three scaled weight copies are concatenated host-side into one
    [D, 192] "wcat" so one matmul pass produces all three Y blocks.

Sharding: rows (nodes) are split across the 8 cores (750 rows each).  Each
core receives its row-slice of features / adjacencies pre-transposed by the
host so every device matmul uses natural (un-transposed) SBUF layouts.  The
small [N, 192] Y and [N, 64] combined activations are all-gathered on-device.
"""

import numpy as np

import concourse.bass as bass
from concourse import bacc
import concourse.mybir as mybir
import concourse.tile as tile
from concourse.bass_utils import run_bass_kernel_spmd

F32 = mybir.dt.float32
F32R = mybir.dt.float32r
F16 = mybir.dt.float16

N_FULL = 6000
D_FULL = 3000
DO = 64
N_CORES = 8


def _chunks(total, step):
    """[(offset, size), ...] covering `total` in steps of `step`."""
    return [(o, min(step, total - o)) for o in range(0, total, step)]


def _pair_groups(chunks):
    """Group consecutive full-128 tiles in pairs (for 2x-sized DMAs)."""
    groups, ii = [], 0
    while ii < len(chunks):
        if (chunks[ii][1] == 128 and ii + 1 < len(chunks)
                and chunks[ii + 1][1] == 128):
            groups.append([(ii, *chunks[ii]), (ii + 1, *chunks[ii + 1])])
            ii += 2
        else:
            groups.append([(ii, *chunks[ii])])
            ii += 1
    return groups


def build_nc(n=N_FULL, d=D_FULL, n_cores=N_CORES, rc_step=384, nd_step=512):
    """Build the SPMD Bass program (identical on every core)."""
    r = n // n_cores  # rows per core
    nc = bacc.Bacc(num_devices=n_cores, num_swdge_queues=4)

    kN = _chunks(n, 128)      # contraction tiles over the node dim
    kD = _chunks(d, 128)      # contraction tiles over the feature dim
    mR = _chunks(r, 128)      # output row tiles (partition dim)
    rcR = _chunks(r, rc_step)  # row chunks (moving free dim)
    nD = _chunks(d, nd_step)   # rec output column chunks

    # ---- I/O ----------------------------------------------------------
    ft = [nc.dram_tensor(f"ft{i}", [d, r], F16, kind="ExternalInput")
          for i in range(3)]
    aspT = [nc.dram_tensor(f"aspT{i}", [n, r], F16, kind="ExternalInput")
            for i in range(3)]
    aftT = [nc.dram_tensor(f"aftT{i}", [n, r], F16, kind="ExternalInput")
            for i in range(3)]
    WC = 3 * DO  # fp16 matmuls run 1 cyc/row at any width: no padding needed
    wcat = [nc.dram_tensor(f"wcat{i}", [d, WC], F16, kind="ExternalInput")
            for i in range(3)]
    wdec = [nc.dram_tensor(f"wdec{i}", [DO, d], F32R, kind="ExternalInput")
            for i in range(3)]
    mw1 = nc.dram_tensor("mw1", [3 * DO, DO], F32R, kind="ExternalInput")
    mb1 = nc.dram_tensor("mb1", [DO, 1], F32, kind="ExternalInput")
    mw2 = nc.dram_tensor("mw2", [DO, DO], F32R, kind="ExternalInput")
    mb2b = nc.dram_tensor("mb2b", [128, DO], F32, kind="ExternalInput")

    latT_out = [nc.dram_tensor(f"latT{i}", [DO, r], F32R, kind="ExternalOutput")
                for i in range(3)]
    comb_out = nc.dram_tensor("comb", [r, DO], F32R, kind="ExternalOutput")
    rec_out = [nc.dram_tensor(f"rec{i}", [r, d], F32R, kind="ExternalOutput")
               for i in range(3)]

    rg = [list(range(n_cores))]

    with tile.TileContext(nc) as tc:
        with (
            tc.tile_pool(name="dram", bufs=1, space="DRAM") as dram,
            tc.tile_pool(name="consts", bufs=1) as consts,
            tc.tile_pool(name="wcat_p", bufs=24) as wcat_p,
            tc.tile_pool(name="fstrip_p", bufs=6) as fstrip_p,
            tc.tile_pool(name="ysb_p", bufs=3) as ysb_p,
            tc.tile_pool(name="ycat_p", bufs=2) as ycat_p,
            tc.tile_pool(name="astrip_p", bufs=8) as astrip_p,
            tc.tile_pool(name="lat_p", bufs=3) as lat_p,
            tc.tile_pool(name="mlp_p", bufs=1) as mlp_p,
            tc.tile_pool(name="csb_p", bufs=4) as csb_p,
            tc.tile_pool(name="cf_p", bufs=1) as cf_p,
            tc.tile_pool(name="wdec_p", bufs=2) as wdec_p,
            tc.tile_pool(name="tsb_p", bufs=2) as tsb_p,
            tc.tile_pool(name="rsb_p", bufs=6) as rsb_p,
            tc.tile_pool(name="psum", bufs=6, space="PSUM") as psum,
            tc.tile_pool(name="psum_s", bufs=2, space="PSUM") as psum_s_p,
        ):
            # internal DRAM for the collectives
            yloc = [dram.tile([r, 3 * DO], F16, name=f"yloc{i}", tag=f"yloc{i}")
                    for i in range(3)]
            yfull = [dram.tile([n, 3 * DO], F16, name=f"yfull{i}",
                               tag=f"yfull{i}", addr_space="Shared")
                     for i in range(3)]
            cloc = dram.tile([r, DO], F16, name="cloc", tag="cloc")
            cfull = dram.tile([n, DO], F16, name="cfull", tag="cfull",
                              addr_space="Shared")

            ones_col = consts.tile([128, 1], F32, name="ones_col")
            nc.vector.memset(ones_col[:], 1.0)

            # ============ Phase A: Y_i = feat_i @ wcat_i, all-gather ======
            for i in range(3):
                wcat_t = []
                for kt, (ko, ksz) in enumerate(kD):
                    wt = wcat_p.tile([128, WC], F16, name="wt",
                                     tag="wcat" if ksz == 128 else "wcat_t",
                                     bufs=24 if ksz == 128 else 2)
                    nc.scalar.dma_start(wt[:ksz, :], wcat[i][ko:ko + ksz, :])
                    wcat_t.append(wt)
                psum_y = [psum.tile([128, WC], F32, name=f"psum_y{m}",
                                    tag="big")
                          for m in range(len(mR))]
                for grp in _pair_groups(kD):
                    if len(grp) == 2:
                        fstrip = fstrip_p.tile([128, 2, r], F16,
                                               name="fstrip", tag="fs2",
                                               bufs=4)
                        nc.gpsimd.dma_start(
                            fstrip[:, :, :],
                            ft[i][grp[0][1]:grp[0][1] + 256, :].rearrange(
                                "(two p) q -> p two q", p=128))
                    else:
                        ksz0 = grp[0][2]
                        fstrip = fstrip_p.tile(
                            [128, 1, r], F16, name="fstrip",
                            tag="fs1" if ksz0 == 128 else "fs_t", bufs=2)
                        nc.gpsimd.dma_start(
                            fstrip[:ksz0, 0, :],
                            ft[i][grp[0][1]:grp[0][1] + ksz0, :])
                    for t, (kt, ko, ksz) in enumerate(grp):
                        for m, (mo, msz) in enumerate(mR):
                            nc.tensor.matmul(
                                psum_y[m][:msz, :],
                                fstrip[:ksz, t, mo:mo + msz],
                                wcat_t[kt][:ksz, :],
                                start=(kt == 0),
                                stop=(kt == len(kD) - 1),
                            )  # psum cols 192:256 are zero padding
                for m, (mo, msz) in enumerate(mR):
                    y_sb = ysb_p.tile([128, 3 * DO], F16, name="y_sb",
                                      tag="y_sb")
                    nc.vector.tensor_copy(out=y_sb[:msz, :],
                                          in_=psum_y[m][:msz, :])
                    nc.scalar.dma_start(yloc[i][mo:mo + msz, :], y_sb[:msz, :])
                nc.gpsimd.collective_compute(
                    "AllGather", mybir.AluOpType.bypass, replica_groups=rg,
                    ins=[yloc[i][:].opt()], outs=[yfull[i][:].opt()],
                )

            # ============ Phase B: latT_i, then MLP -> combined ==========
            lat_sb = []
            for i in range(3):
                ycat_sb = ycat_p.tile([128, len(kN), 3 * DO], F16,
                                      name="ycat_sb", tag="ycat_sb")
                nfull, ntail = n // 128, n % 128
                CH = 8
                engs = [nc.sync, nc.scalar, nc.gpsimd]
                for ci, c0 in enumerate(range(0, nfull, CH)):
                    c1 = min(c0 + CH, nfull)
                    engs[ci % 3].dma_start(
                        ycat_sb[:, c0:c1, :],
                        yfull[i][c0 * 128:c1 * 128, :].rearrange(
                            "(kt p) c -> p kt c", p=128))
                if ntail:
                    zbase = (ntail // 32) * 32
                    nc.vector.memset(ycat_sb[zbase:, nfull, :], 0.0)
                    nc.gpsimd.dma_start(ycat_sb[:ntail, nfull, :],
                                        yfull[i][nfull * 128:, :])
                # column-sum of the b-scaled Y block: DVE-reduce over the
                # k-tile axis, then one ones-matmul to reduce partitions.
                ysum = consts.tile([128, DO], F32, name=f"ysum{i}",
                                   tag=f"ysum{i}")
                nc.vector.reduce_sum(
                    ysum[:, :],
                    ycat_sb[:, :, 2 * DO:3 * DO].rearrange("p k c -> p c k"),
                    axis=mybir.AxisListType.X)
                ps_s = psum_s_p.tile([DO, 1], F32, name="ps_s", tag="s")
                nc.tensor.matmul(ps_s[:, :], ysum[:, :], ones_col[:, :],
                                 start=True, stop=True)
                ps_lat = [psum.tile([DO, 512], F32, name=f"ps_lat{rc}",
                                    tag="big")[:, :rcsz]
                          for rc, (rco, rcsz) in enumerate(rcR)]
                for grp in _pair_groups(kN):
                    if len(grp) == 2:
                        sa = astrip_p.tile([128, 2, r], F16, name="sa",
                                           tag="as2", bufs=6)
                        nc.sync.dma_start(
                            sa[:, :, :],
                            aspT[i][grp[0][1]:grp[0][1] + 256, :].rearrange(
                                "(two p) q -> p two q", p=128))
                        sf = astrip_p.tile([128, 2, r], F16, name="sf",
                                           tag="af2", bufs=6)
                        nc.sync.dma_start(
                            sf[:, :, :],
                            aftT[i][grp[0][1]:grp[0][1] + 256, :].rearrange(
                                "(two p) q -> p two q", p=128))
                    else:
                        ksz0 = grp[0][2]
                        stag = "1" if ksz0 == 128 else "_t"
                        sa = astrip_p.tile([128, 1, r], F16, name="sa",
                                           tag="as" + stag, bufs=2)
                        nc.sync.dma_start(
                            sa[:ksz0, 0, :],
                            aspT[i][grp[0][1]:grp[0][1] + ksz0, :])
                        sf = astrip_p.tile([128, 1, r], F16, name="sf",
                                           tag="af" + stag, bufs=2)
                        nc.sync.dma_start(
                            sf[:ksz0, 0, :],
                            aftT[i][grp[0][1]:grp[0][1] + ksz0, :])
                    for t, (kt, ko, ksz) in enumerate(grp):
                        y0 = ycat_sb[:ksz, kt, 0:DO]
                        y1 = ycat_sb[:ksz, kt, DO:2 * DO]
                        for rc, (rco, rcsz) in enumerate(rcR):
                            nc.tensor.matmul(ps_lat[rc], y0,
                                             sa[:ksz, t, rco:rco + rcsz],
                                             start=(kt == 0), stop=False)
                        for rc, (rco, rcsz) in enumerate(rcR):
                            nc.tensor.matmul(ps_lat[rc], y1,
                                             sf[:ksz, t, rco:rco + rcsz],
                                             start=False,
                                             stop=(kt == len(kN) - 1))
                s_col = consts.tile([DO, 1], F32, name=f"s_col{i}",
                                    tag=f"s_col{i}")
                nc.vector.tensor_copy(out=s_col[:], in_=ps_s[:, :])
                lsb = lat_p.tile([DO, r], F32R, name=f"lat_sb{i}", tag=f"lat{i}")
                for rc, (rco, rcsz) in enumerate(rcR):
                    nc.vector.tensor_tensor(
                        lsb[:, rco:rco + rcsz], ps_lat[rc],
                        s_col[:].to_broadcast([DO, rcsz]),
                        mybir.AluOpType.add,
                    )
                nc.scalar.dma_start(latT_out[i][:, :], lsb[:, :])
                lat_sb.append(lsb)

            # ---- MLP ----
            mw1_sb = mlp_p.tile([DO, 3, DO], F32R, name="mw1_sb", tag="mw1")
            for i in range(3):
                nc.sync.dma_start(mw1_sb[:, i, :], mw1[i * DO:(i + 1) * DO, :])
            mb1_sb = mlp_p.tile([DO, 1], F32, name="mb1_sb", tag="mb1")
            nc.sync.dma_start(mb1_sb[:, :], mb1[:, :])
            mw2_sb = mlp_p.tile([DO, DO], F32R, name="mw2_sb", tag="mw2")
            nc.sync.dma_start(mw2_sb[:, :], mw2[:, :])
            mb2_sb = mlp_p.tile([128, DO], F32, name="mb2_sb", tag="mb2")
            nc.sync.dma_start(mb2_sb[:, :], mb2b[:, :])

            hT_sb = mlp_p.tile([DO, r], F32R, name="hT_sb", tag="hT")
            for rc, (rco, rcsz) in enumerate(rcR):
                ps_h = psum.tile([DO, 512], F32, name="ps_h", tag="big")
                for i in range(3):
                    nc.tensor.matmul(ps_h[:, :rcsz], mw1_sb[:, i, :],
                                     lat_sb[i][:, rco:rco + rcsz],
                                     start=(i == 0), stop=(i == 2))
                nc.vector.tensor_tensor(
                    hT_sb[:, rco:rco + rcsz], ps_h[:, :rcsz],
                    mb1_sb[:].to_broadcast([DO, rcsz]),
                    mybir.AluOpType.add,
                )
            for m, (mo, msz) in enumerate(mR):
                ps_c = psum.tile([128, DO], F32, name="ps_c", tag="big")
                nc.tensor.matmul(ps_c[:msz, :], hT_sb[:, mo:mo + msz],
                                 mw2_sb[:, :], start=True, stop=True)
                c_sb = csb_p.tile([128, DO], F32R, name="c_sb", tag="c_sb")
                nc.vector.tensor_tensor(c_sb[:msz, :], ps_c[:msz, :],
                                        mb2_sb[:msz, :], mybir.AluOpType.add)
                c16 = csb_p.tile([128, DO], F16, name="c16", tag="c16")
                nc.vector.tensor_copy(out=c16[:msz, :], in_=c_sb[:msz, :])
                nc.scalar.dma_start(comb_out[mo:mo + msz, :], c_sb[:msz, :])
                nc.scalar.dma_start(cloc[mo:mo + msz, :], c16[:msz, :])
            nc.gpsimd.collective_compute(
                "AllGather", mybir.AluOpType.bypass, replica_groups=rg,
                ins=[cloc[:].opt()], outs=[cfull[:].opt()],
            )

            # ============ Phase C: rec_i = (A_sp_i @ combined) @ W_dec_i ==
            cf_sb = cf_p.tile([128, len(kN), DO], F16, name="cf_sb", tag="cf")
            nfull, ntail = n // 128, n % 128
            half = nfull // 2
            nc.sync.dma_start(
                cf_sb[:, :half, :],
                cfull[:half * 128, :].rearrange("(kt p) c -> p kt c", p=128))
            nc.scalar.dma_start(
                cf_sb[:, half:nfull, :],
                cfull[half * 128:nfull * 128, :].rearrange(
                    "(kt p) c -> p kt c", p=128))
            if ntail:
                nc.scalar.dma_start(cf_sb[:ntail, nfull, :],
                                    cfull[nfull * 128:, :])
            for i in range(3):
                wdec_sb = wdec_p.tile([DO, d], F32R, name="wdec_sb", tag="wdec")
                nc.scalar.dma_start(wdec_sb[:, :], wdec[i][:, :])
                ps_t = [psum.tile([DO, 512], F32, name=f"ps_t{rc}",
                                  tag="big")[:, :rcsz]
                        for rc, (rco, rcsz) in enumerate(rcR)]
                for gi, grp in enumerate(_pair_groups(kN)):
                    eng = nc.sync if gi % 2 == 0 else nc.scalar
                    if len(grp) == 2:
                        st = astrip_p.tile([128, 2, r], F16, name="st",
                                           tag="as2", bufs=6)
                        eng.dma_start(
                            st[:, :, :],
                            aspT[i][grp[0][1]:grp[0][1] + 256, :].rearrange(
                                "(two p) q -> p two q", p=128))
                    else:
                        ksz0 = grp[0][2]
                        st = astrip_p.tile(
                            [128, 1, r], F16, name="st",
                            tag="as1" if ksz0 == 128 else "as_t", bufs=2)
                        eng.dma_start(st[:ksz0, 0, :],
                                      aspT[i][grp[0][1]:grp[0][1] + ksz0, :])
                    for t, (kt, ko, ksz) in enumerate(grp):
                        for rc, (rco, rcsz) in enumerate(rcR):
                            nc.tensor.matmul(ps_t[rc], cf_sb[:ksz, kt, :],
                                             st[:ksz, t, rco:rco + rcsz],
                                             start=(kt == 0),
                                             stop=(kt == len(kN) - 1))
                t_sb = tsb_p.tile([DO, r], F32R, name="t_sb", tag="t_sb")
                for rc, (rco, rcsz) in enumerate(rcR):
                    nc.vector.tensor_copy(out=t_sb[:, rco:rco + rcsz],
                                          in_=ps_t[rc])
                for m, (mo, msz) in enumerate(mR):
                    for nck, (no, nsz) in enumerate(nD):
                        ps_r = psum.tile([128, 512], F32, name="ps_r",
                                         tag="big")
                        nc.tensor.matmul(ps_r[:msz, :nsz],
                                         t_sb[:, mo:mo + msz],
                                         wdec_sb[:, no:no + nsz],
                                         start=True, stop=True)
                        r_sb = rsb_p.tile([128, 512], F32R, name="r_sb",
                                          tag="r_sb")
                        nc.vector.tensor_copy(out=r_sb[:msz, :nsz],
                                              in_=ps_r[:msz, :nsz])
                        [nc.gpsimd, nc.sync, nc.scalar][
                            (m * len(nD) + nck) % 3].dma_start(
                            rec_out[i][mo:mo + msz, no:no + nsz],
                            r_sb[:msz, :nsz])
    if not nc.is_finalized():
        nc.finalize()
    return nc


def _prep_in_maps(inputs, n, d, n_cores):
    """Host-side sharding: row-slice + transpose the big operands."""
    r = n // n_cores
    feats = [inputs["features_omics1"], inputs["features_omics2"],
             inputs["features_omics3"]]
    asp = [inputs["adj_spatial_omics1"], inputs["adj_spatial_omics2"],
           inputs["adj_spatial_omics3"]]
    aft = [inputs["adj_feature_omics1"], inputs["adj_feature_omics2"],
           inputs["adj_feature_omics3"]]
    convw = [inputs["conv_w1"], inputs["conv_w2"], inputs["conv_w3"]]
    convb = [inputs["conv_b1"], inputs["conv_b2"], inputs["conv_b3"]]
    wenc = [inputs["W_enc1"], inputs["W_enc2"], inputs["W_enc3"]]
    wdec = [inputs["W_dec1"], inputs["W_dec2"], inputs["W_dec3"]]

    f32 = np.float32
    shared = {}
    for i in range(3):
        w0 = f32(np.asarray(convw[i])[0])
        w1 = f32(np.asarray(convw[i])[1])
        b = f32(np.asarray(convb[i]))
        W = np.asarray(wenc[i], f32)
        shared[f"wcat{i}"] = np.ascontiguousarray(
            np.concatenate([w0 * W, w1 * W, b * W], axis=1)).astype(np.float16)
        shared[f"wdec{i}"] = np.ascontiguousarray(np.asarray(wdec[i], f32))
    shared["mw1"] = np.ascontiguousarray(np.asarray(inputs["mlp_w1"], f32))
    shared["mb1"] = np.ascontiguousarray(
        np.asarray(inputs["mlp_b1"], f32).reshape(DO, 1))
    shared["mw2"] = np.ascontiguousarray(np.asarray(inputs["mlp_w2"], f32))
    shared["mb2b"] = np.ascontiguousarray(
        np.tile(np.asarray(inputs["mlp_b2"], f32).reshape(1, DO), (128, 1)))

    in_maps = []
    for c in range(n_cores):
        sl = slice(c * r, (c + 1) * r)
        m = dict(shared)
        for i in range(3):
            m[f"ft{i}"] = np.ascontiguousarray(np.asarray(feats[i][sl], f32).T).astype(np.float16)
            m[f"aspT{i}"] = np.ascontiguousarray(
                np.asarray(asp[i][sl], f32).T).astype(np.float16)
            m[f"aftT{i}"] = np.ascontiguousarray(
                np.asarray(aft[i][sl], f32).T).astype(np.float16)
        in_maps.append(m)
    return in_maps


_NC_CACHE = {}


def kernel(_trace=False, **inputs):
    n, d, n_cores = N_FULL, D_FULL, N_CORES
    in_maps = _prep_in_maps(inputs, n, d, n_cores)
    key = (n, d, n_cores)
    if key not in _NC_CACHE:
        _NC_CACHE[key] = build_nc(n, d, n_cores)
    nc = _NC_CACHE[key]
    res = run_bass_kernel_spmd(nc, in_maps, core_ids=list(range(n_cores)),
                               trace=_trace)
    if _trace:
        kernel._last_results = res
    outs = res.results
    lat = [np.concatenate([outs[c][f"latT{i}"].T for c in range(n_cores)],
                          axis=0)
           for i in range(3)]
    comb = np.concatenate([outs[c]["comb"] for c in range(n_cores)], axis=0)
    rec = [np.concatenate([outs[c][f"rec{i}"] for c in range(n_cores)], axis=0)
           for i in range(3)]
    return (np.ascontiguousarray(lat[0], np.float32),
            np.ascontiguousarray(lat[1], np.float32),
            np.ascontiguousarray(lat[2], np.float32),
            np.ascontiguousarray(comb, np.float32),
            np.ascontiguousarray(rec[0], np.float32),
            np.ascontiguousarray(rec[1], np.float32),
            np.ascontiguousarray(rec[2], np.float32))


# revision 28
# speedup vs baseline: 1.0219x; 1.0048x over previous
"""Trainium2 Bass kernel for the 3-omics GNN encoder (gnn_message_passing).

Math (per reference):
    adj_i   = w0_i*A_sp_i + w1_i*A_ft_i + b_i            (dense fused adjacency)
    lat_i   = adj_i @ (feat_i @ W_enc_i)                 [N, 64]
    combined= (cat(lat) @ mlp_w1 + b1) @ mlp_w2 + b2     [N, 64]
    rec_i   = A_sp_i @ combined @ W_dec_i                [N, D]

Key algebraic optimizations (exact in exact arithmetic):
  * rec_i is computed as (A_sp_i @ combined) @ W_dec_i  -- drops ~216 GFLOP
    per decoder to ~7 GFLOP and makes the problem HBM-bound.
  * the fused adjacency is never materialized:
      lat_i = A_sp_i @ (feat_i @ (w0_i*W_enc_i))
            + A_ft_i @ (feat_i @ (w1_i*W_enc_i))
            + colsum(feat_i @ (b_i*W_enc_i))   (broadcast over rows)
    The three scaled weight copies are concatenated host-side into one
    [D, 192] "wcat" so one matmul pass produces all three Y blocks.

Sharding: rows (nodes) are split across the 8 cores (750 rows each).  Each
core receives its row-slice of features / adjacencies pre-transposed by the
host so every device matmul uses natural (un-transposed) SBUF layouts.  The
small [N, 192] Y and [N, 64] combined activations are all-gathered on-device.
"""

import numpy as np

import concourse.bass as bass
from concourse import bacc
import concourse.mybir as mybir
import concourse.tile as tile
from concourse.bass_utils import run_bass_kernel_spmd

F32 = mybir.dt.float32
F32R = mybir.dt.float32r
F16 = mybir.dt.float16

N_FULL = 6000
D_FULL = 3000
DO = 64
N_CORES = 8


def _chunks(total, step):
    """[(offset, size), ...] covering `total` in steps of `step`."""
    return [(o, min(step, total - o)) for o in range(0, total, step)]


def _pair_groups(chunks):
    """Group consecutive full-128 tiles in pairs (for 2x-sized DMAs)."""
    groups, ii = [], 0
    while ii < len(chunks):
        if (chunks[ii][1] == 128 and ii + 1 < len(chunks)
                and chunks[ii + 1][1] == 128):
            groups.append([(ii, *chunks[ii]), (ii + 1, *chunks[ii + 1])])
            ii += 2
        else:
            groups.append([(ii, *chunks[ii])])
            ii += 1
    return groups


def build_nc(n=N_FULL, d=D_FULL, n_cores=N_CORES, rc_step=384, nd_step=512):
    """Build the SPMD Bass program (identical on every core)."""
    r = n // n_cores  # rows per core
    nc = bacc.Bacc(num_devices=n_cores, num_swdge_queues=4)

    kN = _chunks(n, 128)      # contraction tiles over the node dim
    kD = _chunks(d, 128)      # contraction tiles over the feature dim
    mR = _chunks(r, 128)      # output row tiles (partition dim)
    rcR = _chunks(r, rc_step)  # row chunks (moving free dim)
    nD = _chunks(d, nd_step)   # rec output column chunks

    # ---- I/O ----------------------------------------------------------
    ft = [nc.dram_tensor(f"ft{i}", [d, r], F16, kind="ExternalInput")
          for i in range(3)]
    aspT = [nc.dram_tensor(f"aspT{i}", [n, r], F16, kind="ExternalInput")
            for i in range(3)]
    aftT = [nc.dram_tensor(f"aftT{i}", [n, r], F16, kind="ExternalInput")
            for i in range(3)]
    WC = 3 * DO  # fp16 matmuls run 1 cyc/row at any width: no padding needed
    wcat = [nc.dram_tensor(f"wcat{i}", [d, WC], F16, kind="ExternalInput")
            for i in range(3)]
    wdec = [nc.dram_tensor(f"wdec{i}", [DO, d], F32R, kind="ExternalInput")
            for i in range(3)]
    mw1 = nc.dram_tensor("mw1", [3 * DO, DO], F32R, kind="ExternalInput")
    mb1 = nc.dram_tensor("mb1", [DO, 1], F32, kind="ExternalInput")
    mw2 = nc.dram_tensor("mw2", [DO, DO], F32R, kind="ExternalInput")
    mb2b = nc.dram_tensor("mb2b", [128, DO], F32, kind="ExternalInput")

    latT_out = [nc.dram_tensor(f"latT{i}", [DO, r], F32R, kind="ExternalOutput")
                for i in range(3)]
    comb_out = nc.dram_tensor("comb", [r, DO], F32R, kind="ExternalOutput")
    rec_out = [nc.dram_tensor(f"rec{i}", [r, d], F32R, kind="ExternalOutput")
               for i in range(3)]

    rg = [list(range(n_cores))]

    with tile.TileContext(nc) as tc:
        with (
            tc.tile_pool(name="dram", bufs=1, space="DRAM") as dram,
            tc.tile_pool(name="consts", bufs=1) as consts,
            tc.tile_pool(name="wcat_p", bufs=24) as wcat_p,
            tc.tile_pool(name="fstrip_p", bufs=6) as fstrip_p,
            tc.tile_pool(name="ysb_p", bufs=3) as ysb_p,
            tc.tile_pool(name="ycat_p", bufs=2) as ycat_p,
            tc.tile_pool(name="astrip_p", bufs=8) as astrip_p,
            tc.tile_pool(name="lat_p", bufs=3) as lat_p,
            tc.tile_pool(name="mlp_p", bufs=1) as mlp_p,
            tc.tile_pool(name="csb_p", bufs=4) as csb_p,
            tc.tile_pool(name="cf_p", bufs=1) as cf_p,
            tc.tile_pool(name="wdec_p", bufs=2) as wdec_p,
            tc.tile_pool(name="tsb_p", bufs=2) as tsb_p,
            tc.tile_pool(name="rsb_p", bufs=6) as rsb_p,
            tc.tile_pool(name="psum", bufs=6, space="PSUM") as psum,
            tc.tile_pool(name="psum_s", bufs=2, space="PSUM") as psum_s_p,
        ):
            # internal DRAM for the collectives
            yloc = [dram.tile([r, 3 * DO], F16, name=f"yloc{i}", tag=f"yloc{i}")
                    for i in range(3)]
            yfull = [dram.tile([n, 3 * DO], F16, name=f"yfull{i}",
                               tag=f"yfull{i}", addr_space="Shared")
                     for i in range(3)]
            cloc = dram.tile([r, DO], F16, name="cloc", tag="cloc")
            cfull = dram.tile([n, DO], F16, name="cfull", tag="cfull",
                              addr_space="Shared")

            ones_col = consts.tile([128, 1], F32, name="ones_col")
            nc.vector.memset(ones_col[:], 1.0)

            # ============ Phase A: Y_i = feat_i @ wcat_i, all-gather ======
            for i in range(3):
                wcat_t = []
                for kt, (ko, ksz) in enumerate(kD):
                    wt = wcat_p.tile([128, WC], F16, name="wt",
                                     tag="wcat" if ksz == 128 else "wcat_t",
                                     bufs=24 if ksz == 128 else 2)
                    nc.scalar.dma_start(wt[:ksz, :], wcat[i][ko:ko + ksz, :])
                    wcat_t.append(wt)
                psum_y = [psum.tile([128, WC], F32, name=f"psum_y{m}",
                                    tag="big")
                          for m in range(len(mR))]
                for grp in _pair_groups(kD):
                    if len(grp) == 2:
                        fstrip = fstrip_p.tile([128, 2, r], F16,
                                               name="fstrip", tag="fs2",
                                               bufs=4)
                        nc.gpsimd.dma_start(
                            fstrip[:, :, :],
                            ft[i][grp[0][1]:grp[0][1] + 256, :].rearrange(
                                "(two p) q -> p two q", p=128))
                    else:
                        ksz0 = grp[0][2]
                        fstrip = fstrip_p.tile(
                            [128, 1, r], F16, name="fstrip",
                            tag="fs1" if ksz0 == 128 else "fs_t", bufs=2)
                        nc.gpsimd.dma_start(
                            fstrip[:ksz0, 0, :],
                            ft[i][grp[0][1]:grp[0][1] + ksz0, :])
                    for t, (kt, ko, ksz) in enumerate(grp):
                        for m, (mo, msz) in enumerate(mR):
                            nc.tensor.matmul(
                                psum_y[m][:msz, :],
                                fstrip[:ksz, t, mo:mo + msz],
                                wcat_t[kt][:ksz, :],
                                start=(kt == 0),
                                stop=(kt == len(kD) - 1),
                            )  # psum cols 192:256 are zero padding
                for m, (mo, msz) in enumerate(mR):
                    y_sb = ysb_p.tile([128, 3 * DO], F16, name="y_sb",
                                      tag="y_sb")
                    nc.vector.tensor_copy(out=y_sb[:msz, :],
                                          in_=psum_y[m][:msz, :])
                    nc.scalar.dma_start(yloc[i][mo:mo + msz, :], y_sb[:msz, :])
                nc.gpsimd.collective_compute(
                    "AllGather", mybir.AluOpType.bypass, replica_groups=rg,
                    ins=[yloc[i][:].opt()], outs=[yfull[i][:].opt()],
                )

            # ============ Phase B: latT_i, then MLP -> combined ==========
            lat_sb = []
            for i in range(3):
                ycat_sb = ycat_p.tile([128, len(kN), 3 * DO], F16,
                                      name="ycat_sb", tag="ycat_sb")
                nfull, ntail = n // 128, n % 128
                CH = 8
                engs = [nc.sync, nc.scalar, nc.gpsimd]
                for ci, c0 in enumerate(range(0, nfull, CH)):
                    c1 = min(c0 + CH, nfull)
                    engs[ci % 3].dma_start(
                        ycat_sb[:, c0:c1, :],
                        yfull[i][c0 * 128:c1 * 128, :].rearrange(
                            "(kt p) c -> p kt c", p=128))
                if ntail:
                    zbase = (ntail // 32) * 32
                    nc.vector.memset(ycat_sb[zbase:, nfull, :], 0.0)
                    nc.gpsimd.dma_start(ycat_sb[:ntail, nfull, :],
                                        yfull[i][nfull * 128:, :])
                # column-sum of the b-scaled Y block: DVE-reduce over the
                # k-tile axis, then one ones-matmul to reduce partitions.
                ysum = consts.tile([128, DO], F32, name=f"ysum{i}",
                                   tag=f"ysum{i}")
                nc.vector.reduce_sum(
                    ysum[:, :],
                    ycat_sb[:, :, 2 * DO:3 * DO].rearrange("p k c -> p c k"),
                    axis=mybir.AxisListType.X)
                ps_s = psum_s_p.tile([DO, 1], F32, name="ps_s", tag="s")
                nc.tensor.matmul(ps_s[:, :], ysum[:, :], ones_col[:, :],
                                 start=True, stop=True)
                ps_lat = [psum.tile([DO, 512], F32, name=f"ps_lat{rc}",
                                    tag="big")[:, :rcsz]
                          for rc, (rco, rcsz) in enumerate(rcR)]
                for grp in _pair_groups(kN):
                    if len(grp) == 2:
                        sa = astrip_p.tile([128, 2, r], F16, name="sa",
                                           tag="as2", bufs=6)
                        nc.sync.dma_start(
                            sa[:, :, :],
                            aspT[i][grp[0][1]:grp[0][1] + 256, :].rearrange(
                                "(two p) q -> p two q", p=128))
                        sf = astrip_p.tile([128, 2, r], F16, name="sf",
                                           tag="af2", bufs=6)
                        nc.scalar.dma_start(
                            sf[:, :, :],
                            aftT[i][grp[0][1]:grp[0][1] + 256, :].rearrange(
                                "(two p) q -> p two q", p=128))
                    else:
                        ksz0 = grp[0][2]
                        stag = "1" if ksz0 == 128 else "_t"
                        sa = astrip_p.tile([128, 1, r], F16, name="sa",
                                           tag="as" + stag, bufs=2)
                        nc.sync.dma_start(
                            sa[:ksz0, 0, :],
                            aspT[i][grp[0][1]:grp[0][1] + ksz0, :])
                        sf = astrip_p.tile([128, 1, r], F16, name="sf",
                                           tag="af" + stag, bufs=2)
                        nc.scalar.dma_start(
                            sf[:ksz0, 0, :],
                            aftT[i][grp[0][1]:grp[0][1] + ksz0, :])
                    for t, (kt, ko, ksz) in enumerate(grp):
                        y0 = ycat_sb[:ksz, kt, 0:DO]
                        y1 = ycat_sb[:ksz, kt, DO:2 * DO]
                        for rc, (rco, rcsz) in enumerate(rcR):
                            nc.tensor.matmul(ps_lat[rc], y0,
                                             sa[:ksz, t, rco:rco + rcsz],
                                             start=(kt == 0), stop=False)
                        for rc, (rco, rcsz) in enumerate(rcR):
                            nc.tensor.matmul(ps_lat[rc], y1,
                                             sf[:ksz, t, rco:rco + rcsz],
                                             start=False,
                                             stop=(kt == len(kN) - 1))
                s_col = consts.tile([DO, 1], F32, name=f"s_col{i}",
                                    tag=f"s_col{i}")
                nc.vector.tensor_copy(out=s_col[:], in_=ps_s[:, :])
                lsb = lat_p.tile([DO, r], F32R, name=f"lat_sb{i}", tag=f"lat{i}")
                for rc, (rco, rcsz) in enumerate(rcR):
                    nc.vector.tensor_tensor(
                        lsb[:, rco:rco + rcsz], ps_lat[rc],
                        s_col[:].to_broadcast([DO, rcsz]),
                        mybir.AluOpType.add,
                    )
                nc.scalar.dma_start(latT_out[i][:, :], lsb[:, :])
                lat_sb.append(lsb)

            # ---- MLP ----
            mw1_sb = mlp_p.tile([DO, 3, DO], F32R, name="mw1_sb", tag="mw1")
            for i in range(3):
                nc.sync.dma_start(mw1_sb[:, i, :], mw1[i * DO:(i + 1) * DO, :])
            mb1_sb = mlp_p.tile([DO, 1], F32, name="mb1_sb", tag="mb1")
            nc.sync.dma_start(mb1_sb[:, :], mb1[:, :])
            mw2_sb = mlp_p.tile([DO, DO], F32R, name="mw2_sb", tag="mw2")
            nc.sync.dma_start(mw2_sb[:, :], mw2[:, :])
            mb2_sb = mlp_p.tile([128, DO], F32, name="mb2_sb", tag="mb2")
            nc.sync.dma_start(mb2_sb[:, :], mb2b[:, :])

            hT_sb = mlp_p.tile([DO, r], F32R, name="hT_sb", tag="hT")
            for rc, (rco, rcsz) in enumerate(rcR):
                ps_h = psum.tile([DO, 512], F32, name="ps_h", tag="big")
                for i in range(3):
                    nc.tensor.matmul(ps_h[:, :rcsz], mw1_sb[:, i, :],
                                     lat_sb[i][:, rco:rco + rcsz],
                                     start=(i == 0), stop=(i == 2))
                nc.vector.tensor_tensor(
                    hT_sb[:, rco:rco + rcsz], ps_h[:, :rcsz],
                    mb1_sb[:].to_broadcast([DO, rcsz]),
                    mybir.AluOpType.add,
                )
            for m, (mo, msz) in enumerate(mR):
                ps_c = psum.tile([128, DO], F32, name="ps_c", tag="big")
                nc.tensor.matmul(ps_c[:msz, :], hT_sb[:, mo:mo + msz],
                                 mw2_sb[:, :], start=True, stop=True)
                c_sb = csb_p.tile([128, DO], F32R, name="c_sb", tag="c_sb")
                nc.vector.tensor_tensor(c_sb[:msz, :], ps_c[:msz, :],
                                        mb2_sb[:msz, :], mybir.AluOpType.add)
                c16 = csb_p.tile([128, DO], F16, name="c16", tag="c16")
                nc.vector.tensor_copy(out=c16[:msz, :], in_=c_sb[:msz, :])
                nc.scalar.dma_start(comb_out[mo:mo + msz, :], c_sb[:msz, :])
                nc.scalar.dma_start(cloc[mo:mo + msz, :], c16[:msz, :])
            nc.gpsimd.collective_compute(
                "AllGather", mybir.AluOpType.bypass, replica_groups=rg,
                ins=[cloc[:].opt()], outs=[cfull[:].opt()],
            )

            # ============ Phase C: rec_i = (A_sp_i @ combined) @ W_dec_i ==
            cf_sb = cf_p.tile([128, len(kN), DO], F16, name="cf_sb", tag="cf")
            nfull, ntail = n // 128, n % 128
            half = nfull // 2
            nc.sync.dma_start(
                cf_sb[:, :half, :],
                cfull[:half * 128, :].rearrange("(kt p) c -> p kt c", p=128))
            nc.scalar.dma_start(
                cf_sb[:, half:nfull, :],
                cfull[half * 128:nfull * 128, :].rearrange(
                    "(kt p) c -> p kt c", p=128))
            if ntail:
                nc.scalar.dma_start(cf_sb[:ntail, nfull, :],
                                    cfull[nfull * 128:, :])
            for i in range(3):
                wdec_sb = wdec_p.tile([DO, d], F32R, name="wdec_sb", tag="wdec")
                nc.scalar.dma_start(wdec_sb[:, :], wdec[i][:, :])
                ps_t = [psum.tile([DO, 512], F32, name=f"ps_t{rc}",
                                  tag="big")[:, :rcsz]
                        for rc, (rco, rcsz) in enumerate(rcR)]
                for gi, grp in enumerate(_pair_groups(kN)):
                    eng = nc.sync if gi % 2 == 0 else nc.scalar
                    if len(grp) == 2:
                        st = astrip_p.tile([128, 2, r], F16, name="st",
                                           tag="as2", bufs=6)
                        eng.dma_start(
                            st[:, :, :],
                            aspT[i][grp[0][1]:grp[0][1] + 256, :].rearrange(
                                "(two p) q -> p two q", p=128))
                    else:
                        ksz0 = grp[0][2]
                        st = astrip_p.tile(
                            [128, 1, r], F16, name="st",
                            tag="as1" if ksz0 == 128 else "as_t", bufs=2)
                        eng.dma_start(st[:ksz0, 0, :],
                                      aspT[i][grp[0][1]:grp[0][1] + ksz0, :])
                    for t, (kt, ko, ksz) in enumerate(grp):
                        for rc, (rco, rcsz) in enumerate(rcR):
                            nc.tensor.matmul(ps_t[rc], cf_sb[:ksz, kt, :],
                                             st[:ksz, t, rco:rco + rcsz],
                                             start=(kt == 0),
                                             stop=(kt == len(kN) - 1))
                t_sb = tsb_p.tile([DO, r], F32R, name="t_sb", tag="t_sb")
                for rc, (rco, rcsz) in enumerate(rcR):
                    nc.vector.tensor_copy(out=t_sb[:, rco:rco + rcsz],
                                          in_=ps_t[rc])
                for m, (mo, msz) in enumerate(mR):
                    for nck, (no, nsz) in enumerate(nD):
                        ps_r = psum.tile([128, 512], F32, name="ps_r",
                                         tag="big")
                        nc.tensor.matmul(ps_r[:msz, :nsz],
                                         t_sb[:, mo:mo + msz],
                                         wdec_sb[:, no:no + nsz],
                                         start=True, stop=True)
                        r_sb = rsb_p.tile([128, 512], F32R, name="r_sb",
                                          tag="r_sb")
                        nc.vector.tensor_copy(out=r_sb[:msz, :nsz],
                                              in_=ps_r[:msz, :nsz])
                        [nc.gpsimd, nc.sync, nc.scalar][
                            (m * len(nD) + nck) % 3].dma_start(
                            rec_out[i][mo:mo + msz, no:no + nsz],
                            r_sb[:msz, :nsz])
    if not nc.is_finalized():
        nc.finalize()
    return nc


def _prep_in_maps(inputs, n, d, n_cores):
    """Host-side sharding: row-slice + transpose the big operands."""
    r = n // n_cores
    feats = [inputs["features_omics1"], inputs["features_omics2"],
             inputs["features_omics3"]]
    asp = [inputs["adj_spatial_omics1"], inputs["adj_spatial_omics2"],
           inputs["adj_spatial_omics3"]]
    aft = [inputs["adj_feature_omics1"], inputs["adj_feature_omics2"],
           inputs["adj_feature_omics3"]]
    convw = [inputs["conv_w1"], inputs["conv_w2"], inputs["conv_w3"]]
    convb = [inputs["conv_b1"], inputs["conv_b2"], inputs["conv_b3"]]
    wenc = [inputs["W_enc1"], inputs["W_enc2"], inputs["W_enc3"]]
    wdec = [inputs["W_dec1"], inputs["W_dec2"], inputs["W_dec3"]]

    f32 = np.float32
    shared = {}
    for i in range(3):
        w0 = f32(np.asarray(convw[i])[0])
        w1 = f32(np.asarray(convw[i])[1])
        b = f32(np.asarray(convb[i]))
        W = np.asarray(wenc[i], f32)
        shared[f"wcat{i}"] = np.ascontiguousarray(
            np.concatenate([w0 * W, w1 * W, b * W], axis=1)).astype(np.float16)
        shared[f"wdec{i}"] = np.ascontiguousarray(np.asarray(wdec[i], f32))
    shared["mw1"] = np.ascontiguousarray(np.asarray(inputs["mlp_w1"], f32))
    shared["mb1"] = np.ascontiguousarray(
        np.asarray(inputs["mlp_b1"], f32).reshape(DO, 1))
    shared["mw2"] = np.ascontiguousarray(np.asarray(inputs["mlp_w2"], f32))
    shared["mb2b"] = np.ascontiguousarray(
        np.tile(np.asarray(inputs["mlp_b2"], f32).reshape(1, DO), (128, 1)))

    in_maps = []
    for c in range(n_cores):
        sl = slice(c * r, (c + 1) * r)
        m = dict(shared)
        for i in range(3):
            m[f"ft{i}"] = np.ascontiguousarray(np.asarray(feats[i][sl], f32).T).astype(np.float16)
            m[f"aspT{i}"] = np.ascontiguousarray(
                np.asarray(asp[i][sl], f32).T).astype(np.float16)
            m[f"aftT{i}"] = np.ascontiguousarray(
                np.asarray(aft[i][sl], f32).T).astype(np.float16)
        in_maps.append(m)
    return in_maps


_NC_CACHE = {}


def kernel(_trace=False, **inputs):
    n, d, n_cores = N_FULL, D_FULL, N_CORES
    in_maps = _prep_in_maps(inputs, n, d, n_cores)
    key = (n, d, n_cores)
    if key not in _NC_CACHE:
        _NC_CACHE[key] = build_nc(n, d, n_cores)
    nc = _NC_CACHE[key]
    res = run_bass_kernel_spmd(nc, in_maps, core_ids=list(range(n_cores)),
                               trace=_trace)
    if _trace:
        kernel._last_results = res
    outs = res.results
    lat = [np.concatenate([outs[c][f"latT{i}"].T for c in range(n_cores)],
                          axis=0)
           for i in range(3)]
    comb = np.concatenate([outs[c]["comb"] for c in range(n_cores)], axis=0)
    rec = [np.concatenate([outs[c][f"rec{i}"] for c in range(n_cores)], axis=0)
           for i in range(3)]
    return (np.ascontiguousarray(lat[0], np.float32),
            np.ascontiguousarray(lat[1], np.float32),
            np.ascontiguousarray(lat[2], np.float32),
            np.ascontiguousarray(comb, np.float32),
            np.ascontiguousarray(rec[0], np.float32),
            np.ascontiguousarray(rec[1], np.float32),
            np.ascontiguousarray(rec[2], np.float32))
